# revision 2
# baseline (speedup 1.0000x reference)
"""Per-sample dynamic 3x3 convolution (B=16, C=128, 64x64, pad 1) on 8
Trainium2 NeuronCores.

Sharding: pure data parallel — batch 16 -> 2 samples per core, no
cross-core communication.

Device kernel (per core, per sample), implicit GEMM in bf16:
  - features are host zero-padded to (128ci, 66*66) and cast to bf16 so
    every DMA is contiguous and half the fp32 bytes; the dynamic kernel
    is host pre-transposed to (128ci, tap*co) bf16 so each tap's
    (ci, co) slice is a matmul lhsT.  bf16 matmuls stream the moving
    operand at 1 col/cycle (same as float32r) but halve all HBM traffic;
    measured end-to-end rel l2 err ~2.9e-3 (PSUM accumulates fp32).
  - output rows are produced 8 at a time (512 px = one PSUM bank);
    each chunk accumulates 9 bf16 matmuls (one per 3x3 tap) with
    shifted windows into the padded image.
  - PSUM is evacuated by VectorE copies converting to bf16 (halves
    store bytes; host upcasts to fp32).
  - feature row-slices ALTERNATE between the sync and scalar HWDGE
    rings (2x aggregate DMA queue throughput; ~1KB/partition slices run
    well under the per-ring peak); weights and output stores share the
    scalar ring, all input DMAs issued before any store so the FIFO
    rings can't head-of-line-block loads behind compute-gated stores.
  - the feature image is loaded in 8 row-slices (10-row first slice)
    so chunk-0 matmuls start after ~1/7 of the image has landed.
"""

from contextlib import ExitStack

import numpy as np

B = 16
N_CORES = 8
BPC = B // N_CORES  # samples per core
CI = 128
CO = 128
H = W = 64
KS = 3
PADW = W + 2
PADH = H + 2
NPIX = H * W
ROWS_PER_CHUNK = 8
NCHUNK = H // ROWS_PER_CHUNK
NFREE = ROWS_PER_CHUNK * W  # 512 = one PSUM bank of fp32

FEAT_SPLIT = 8

_CACHE = {}


def _build_conv():
    import concourse.tile as tile
    from concourse import bacc, mybir

    F32 = mybir.dt.float32
    BF16 = mybir.dt.bfloat16

    nc = bacc.Bacc("TRN2", target_bir_lowering=False, debug=False,
                   num_devices=N_CORES)
    feats = nc.dram_tensor("features", [BPC, CI, PADH * PADW], BF16,
                           kind="ExternalInput").ap()
    wts = nc.dram_tensor("weights", [BPC, CI, KS * KS * CO], BF16,
                         kind="ExternalInput").ap()
    out = nc.dram_tensor("out", [BPC, CO, NPIX], BF16,
                         kind="ExternalOutput").ap()

    with tile.TileContext(nc) as tc:
        with ExitStack() as ctx:
            xpool = ctx.enter_context(tc.tile_pool(name="xpad", bufs=2))
            wpool = ctx.enter_context(tc.tile_pool(name="wts", bufs=2))
            opool = ctx.enter_context(tc.tile_pool(name="outb", bufs=4))
            pspool = ctx.enter_context(
                tc.tile_pool(name="psum", bufs=8, space="PSUM"))

            # Issue every input DMA before any output store enters the
            # HWDGE rings: rings are FIFO per issuing engine, so an
            # out-DMA waiting on its PSUM-evacuation copy would head-of-
            # line-block sample 1's weight/feature loads.
            wt_tiles, xp_views = {}, {}
            for b in range(BPC):
                wt = wpool.tile([CI, KS * KS * CO], BF16, tag="wt",
                                name=f"wt{b}")
                if b == 0:
                    # tap-0 slice first: the very first matmul needs only
                    # wt[:, :CO]
                    nc.scalar.dma_start(wt[:, :CO], wts[b][:, :CO])
                    nc.scalar.dma_start(wt[:, CO:], wts[b][:, CO:])
                else:
                    nc.scalar.dma_start(wt[:], wts[b])
                wt_tiles[b] = wt
            splits = np.linspace(0, PADH, FEAT_SPLIT + 1).astype(int)
            for b in range(BPC):
                xp = xpool.tile([CI, PADH * PADW], BF16, tag="xp",
                                name=f"xp{b}")
                xpv = xp[:].rearrange("p (h w) -> p h w", w=PADW)
                fv = feats[b].rearrange("p (h w) -> p h w", w=PADW)
                if b == 0:
                    # 10-row first slice: covers chunk 0's padded rows
                    # 0..9 so its matmuls unblock after ~1/7 of the image
                    bounds = [0, 10] + list(
                        np.linspace(10, PADH, FEAT_SPLIT).astype(int))[1:]
                else:
                    bounds = list(splits)
                for i, (s0, s1) in enumerate(zip(bounds[:-1], bounds[1:])):
                    # alternate slices across both HWDGE rings for 2x
                    # aggregate DMA-queue throughput
                    eng = nc.sync if (i + b) % 2 == 0 else nc.scalar
                    eng.dma_start(xpv[:, s0:s1, :], fv[:, s0:s1, :])
                xp_views[b] = xpv

            for b in range(BPC):
                wt = wt_tiles[b]
                xpv = xp_views[b]
                for k in range(NCHUNK):
                    ps = pspool.tile([CO, NFREE], F32, tag="ps",
                                     name=f"ps{b}_{k}")
                    for t in range(KS * KS):
                        kh, kw = divmod(t, KS)
                        r0 = ROWS_PER_CHUNK * k + kh
                        rhs = xpv[:, r0:r0 + ROWS_PER_CHUNK, kw:kw + W]
                        nc.tensor.matmul(ps[:], wt[:, t * CO:(t + 1) * CO],
                                         rhs, start=(t == 0),
                                         stop=(t == KS * KS - 1))
                    ob = opool.tile([CO, NFREE], BF16)
                    nc.vector.tensor_copy(ob[:], ps[:])
                    nc.scalar.dma_start(out[b][:, NFREE * k:NFREE * (k + 1)],
                                        ob[:])
    nc.compile()
    return nc


def _host_pack_weights(dynamic_kernel):
    import ml_dtypes
    w = np.ascontiguousarray(
        np.asarray(dynamic_kernel).astype(np.float32).transpose(0, 2, 3, 4, 1))
    return w.reshape(B, CI, KS * KS * CO).astype(ml_dtypes.bfloat16)


def _host_pad_features(features):
    import ml_dtypes
    xp = np.zeros((B, CI, PADH, PADW), ml_dtypes.bfloat16)
    xp[:, :, 1:H + 1, 1:W + 1] = np.asarray(features).astype(
        ml_dtypes.bfloat16)
    return xp.reshape(B, CI, PADH * PADW)


def kernel(features, dynamic_kernel):
    """features (16,128,64,64) f32, dynamic_kernel (16,128,128,3,3) f32
    -> (16,128,64,64) f32."""
    from concourse.bass_utils import run_bass_kernel_spmd

    features = np.asarray(features)
    dynamic_kernel = np.asarray(dynamic_kernel)

    if "nc" not in _CACHE:
        _CACHE["nc"] = _build_conv()
    nc = _CACHE["nc"]

    f_padded = _host_pad_features(features)
    w_packed = _host_pack_weights(dynamic_kernel)
    in_maps = [{"features": f_padded[BPC * c:BPC * (c + 1)],
                "weights": w_packed[BPC * c:BPC * (c + 1)]}
               for c in range(N_CORES)]

    import time as _time
    last_err = None
    for attempt in range(4):  # transient NRT/device errors: retry
        try:
            res = run_bass_kernel_spmd(nc, in_maps,
                                       core_ids=list(range(N_CORES)))
            break
        except Exception as e:  # noqa: BLE001
            last_err = e
            # give the terminal time to recover a wedged core before
            # the next attempt (immediate retries hit the same state)
            _time.sleep(5 * (attempt + 1))
    else:
        raise last_err

    got = np.concatenate([res.results[c]["out"] for c in range(N_CORES)],
                         axis=0)
    return got.reshape(B, CO, H, W).astype(np.float32)


# revision 3
# speedup vs baseline: 1.0615x; 1.0615x over previous
"""Per-sample dynamic 3x3 convolution (B=16, C=128, 64x64, pad 1) on 8
Trainium2 NeuronCores.

Sharding: pure data parallel — batch 16 -> 2 samples per core, no
cross-core communication.

Device kernel (per core, per sample), implicit GEMM in bf16:
  - features are host zero-padded to (128ci, 66*66) and cast to bf16 so
    every DMA is contiguous and half the fp32 bytes; the dynamic kernel
    is host pre-transposed to (128ci, tap*co) bf16 so each tap's
    (ci, co) slice is a matmul lhsT.  bf16 matmuls stream the moving
    operand at 1 col/cycle (same as float32r) but halve all HBM traffic;
    measured end-to-end rel l2 err ~2.9e-3 (PSUM accumulates fp32).
  - output rows are produced 8 at a time (512 px = one PSUM bank);
    each chunk accumulates 9 bf16 matmuls (one per 3x3 tap) with
    shifted windows into the padded image.
  - PSUM is evacuated by VectorE copies converting to bf16 (halves
    store bytes; host upcasts to fp32).
  - sample 0's feature row-slices stream on the sync HWDGE ring while
    sample 1's stream on the scalar ring (2x aggregate DMA queue
    throughput; a single ring runs ~140 GB/s with these ~1KB/partition
    slices); weights and output stores share the scalar ring, all input
    DMAs issued before any store so the FIFO rings can't head-of-line-
    block loads behind compute-gated stores.
  - the feature image is loaded in 8 row-slices (10-row first slice)
    so chunk-0 matmuls start after ~1/7 of the image has landed.
"""

from contextlib import ExitStack

import numpy as np

B = 16
N_CORES = 8
BPC = B // N_CORES  # samples per core
CI = 128
CO = 128
H = W = 64
KS = 3
PADW = W + 2
PADH = H + 2
NPIX = H * W
ROWS_PER_CHUNK = 8
NCHUNK = H // ROWS_PER_CHUNK
NFREE = ROWS_PER_CHUNK * W  # 512 = one PSUM bank of fp32

FEAT_SPLIT = 8

_CACHE = {}


def _build_conv():
    import concourse.tile as tile
    from concourse import bacc, mybir

    F32 = mybir.dt.float32
    BF16 = mybir.dt.bfloat16

    nc = bacc.Bacc("TRN2", target_bir_lowering=False, debug=False,
                   num_devices=N_CORES)
    feats = nc.dram_tensor("features", [BPC, CI, PADH * PADW], BF16,
                           kind="ExternalInput").ap()
    wts = nc.dram_tensor("weights", [BPC, CI, KS * KS * CO], BF16,
                         kind="ExternalInput").ap()
    out = nc.dram_tensor("out", [BPC, CO, NPIX], BF16,
                         kind="ExternalOutput").ap()

    with tile.TileContext(nc) as tc:
        with ExitStack() as ctx:
            xpool = ctx.enter_context(tc.tile_pool(name="xpad", bufs=2))
            wpool = ctx.enter_context(tc.tile_pool(name="wts", bufs=2))
            opool = ctx.enter_context(tc.tile_pool(name="outb", bufs=4))
            pspool = ctx.enter_context(
                tc.tile_pool(name="psum", bufs=8, space="PSUM"))

            # Issue every input DMA before any output store enters the
            # HWDGE rings: rings are FIFO per issuing engine, so an
            # out-DMA waiting on its PSUM-evacuation copy would head-of-
            # line-block sample 1's weight/feature loads.
            wt_tiles, xp_views = {}, {}
            for b in range(BPC):
                wt = wpool.tile([CI, KS * KS * CO], BF16, tag="wt",
                                name=f"wt{b}")
                if b == 0:
                    # tap-0 slice first: the very first matmul needs only
                    # wt[:, :CO]
                    nc.scalar.dma_start(wt[:, :CO], wts[b][:, :CO])
                    nc.scalar.dma_start(wt[:, CO:], wts[b][:, CO:])
                else:
                    nc.scalar.dma_start(wt[:], wts[b])
                wt_tiles[b] = wt
            splits = np.linspace(0, PADH, FEAT_SPLIT + 1).astype(int)
            for b in range(BPC):
                xp = xpool.tile([CI, PADH * PADW], BF16, tag="xp",
                                name=f"xp{b}")
                xpv = xp[:].rearrange("p (h w) -> p h w", w=PADW)
                fv = feats[b].rearrange("p (h w) -> p h w", w=PADW)
                if b == 0:
                    # 10-row first slice: covers chunk 0's padded rows
                    # 0..9 so its matmuls unblock after ~1/7 of the image
                    bounds = [0, 10] + list(
                        np.linspace(10, PADH, FEAT_SPLIT).astype(int))[1:]
                else:
                    bounds = list(splits)
                for s0, s1 in zip(bounds[:-1], bounds[1:]):
                    # sample 0 -> sync ring, sample 1 -> scalar ring: the
                    # two HWDGE queues stream in parallel (2x aggregate
                    # throughput) and sample 1 is fully resident well
                    # before the PE reaches it
                    eng = nc.sync if b == 0 else nc.scalar
                    eng.dma_start(xpv[:, s0:s1, :], fv[:, s0:s1, :])
                xp_views[b] = xpv

            for b in range(BPC):
                wt = wt_tiles[b]
                xpv = xp_views[b]
                for k in range(NCHUNK):
                    ps = pspool.tile([CO, NFREE], F32, tag="ps",
                                     name=f"ps{b}_{k}")
                    for t in range(KS * KS):
                        kh, kw = divmod(t, KS)
                        r0 = ROWS_PER_CHUNK * k + kh
                        rhs = xpv[:, r0:r0 + ROWS_PER_CHUNK, kw:kw + W]
                        nc.tensor.matmul(ps[:], wt[:, t * CO:(t + 1) * CO],
                                         rhs, start=(t == 0),
                                         stop=(t == KS * KS - 1))
                    ob = opool.tile([CO, NFREE], BF16)
                    nc.vector.tensor_copy(ob[:], ps[:])
                    nc.scalar.dma_start(out[b][:, NFREE * k:NFREE * (k + 1)],
                                        ob[:])
    nc.compile()
    return nc


def _host_pack_weights(dynamic_kernel):
    import ml_dtypes
    w = np.ascontiguousarray(
        np.asarray(dynamic_kernel).astype(np.float32).transpose(0, 2, 3, 4, 1))
    return w.reshape(B, CI, KS * KS * CO).astype(ml_dtypes.bfloat16)


def _host_pad_features(features):
    import ml_dtypes
    xp = np.zeros((B, CI, PADH, PADW), ml_dtypes.bfloat16)
    xp[:, :, 1:H + 1, 1:W + 1] = np.asarray(features).astype(
        ml_dtypes.bfloat16)
    return xp.reshape(B, CI, PADH * PADW)


def kernel(features, dynamic_kernel):
    """features (16,128,64,64) f32, dynamic_kernel (16,128,128,3,3) f32
    -> (16,128,64,64) f32."""
    from concourse.bass_utils import run_bass_kernel_spmd

    features = np.asarray(features)
    dynamic_kernel = np.asarray(dynamic_kernel)

    if "nc" not in _CACHE:
        _CACHE["nc"] = _build_conv()
    nc = _CACHE["nc"]

    f_padded = _host_pad_features(features)
    w_packed = _host_pack_weights(dynamic_kernel)
    in_maps = [{"features": f_padded[BPC * c:BPC * (c + 1)],
                "weights": w_packed[BPC * c:BPC * (c + 1)]}
               for c in range(N_CORES)]

    import time as _time
    last_err = None
    for attempt in range(4):  # transient NRT/device errors: retry
        try:
            res = run_bass_kernel_spmd(nc, in_maps,
                                       core_ids=list(range(N_CORES)))
            break
        except Exception as e:  # noqa: BLE001
            last_err = e
            # give the terminal time to recover a wedged core before
            # the next attempt (immediate retries hit the same state)
            _time.sleep(5 * (attempt + 1))
    else:
        raise last_err

    got = np.concatenate([res.results[c]["out"] for c in range(N_CORES)],
                         axis=0)
    return got.reshape(B, CO, H, W).astype(np.float32)


# revision 4
# speedup vs baseline: 1.0832x; 1.0204x over previous
"""Per-sample dynamic 3x3 convolution (B=16, C=128, 64x64, pad 1) on 8
Trainium2 NeuronCores.

Sharding: pure data parallel — batch 16 -> 2 samples per core, no
cross-core communication.

Device kernel (per core, per sample), implicit GEMM in bf16:
  - features are host zero-padded to (128ci, 66*66) and cast to bf16 so
    every DMA is contiguous and half the fp32 bytes; the dynamic kernel
    is host pre-transposed to (128ci, tap*co) bf16 so each tap's
    (ci, co) slice is a matmul lhsT.  bf16 matmuls stream the moving
    operand at 1 col/cycle (same as float32r) but halve all HBM traffic;
    measured end-to-end rel l2 err ~2.9e-3 (PSUM accumulates fp32).
  - output rows are produced 8 at a time (512 px = one PSUM bank);
    each chunk accumulates 9 bf16 matmuls (one per 3x3 tap) with
    shifted windows into the padded image.
  - PSUM is evacuated by VectorE copies converting to bf16 (halves
    store bytes; host upcasts to fp32).
  - sample 0's feature row-slices stream on the sync HWDGE ring while
    sample 1's stream on the scalar ring (2x aggregate DMA queue
    throughput; a single ring runs ~140 GB/s with these ~1KB/partition
    slices); weights and output stores share the scalar ring, all input
    DMAs issued before any store so the FIFO rings can't head-of-line-
    block loads behind compute-gated stores.
  - the feature image is loaded in 8 row-slices (10-row first slice)
    so chunk-0 matmuls start after ~1/7 of the image has landed.
"""

from contextlib import ExitStack

import numpy as np

B = 16
N_CORES = 8
BPC = B // N_CORES  # samples per core
CI = 128
CO = 128
H = W = 64
KS = 3
PADW = W + 2
PADH = H + 2
NPIX = H * W
ROWS_PER_CHUNK = 8
NCHUNK = H // ROWS_PER_CHUNK
NFREE = ROWS_PER_CHUNK * W  # 512 = one PSUM bank of fp32

FEAT_SPLIT = 8

_CACHE = {}


def _build_conv():
    import concourse.tile as tile
    from concourse import bacc, mybir

    F32 = mybir.dt.float32
    BF16 = mybir.dt.bfloat16

    nc = bacc.Bacc("TRN2", target_bir_lowering=False, debug=False,
                   num_devices=N_CORES)
    feats = nc.dram_tensor("features", [BPC, CI, PADH * PADW], BF16,
                           kind="ExternalInput").ap()
    wts = nc.dram_tensor("weights", [BPC, CI, KS * KS * CO], BF16,
                         kind="ExternalInput").ap()
    out = nc.dram_tensor("out", [BPC, CO, NPIX], BF16,
                         kind="ExternalOutput").ap()

    with tile.TileContext(nc) as tc:
        with ExitStack() as ctx:
            xpool = ctx.enter_context(tc.tile_pool(name="xpad", bufs=2))
            wpool = ctx.enter_context(tc.tile_pool(name="wts", bufs=2))
            opool = ctx.enter_context(tc.tile_pool(name="outb", bufs=4))
            pspool = ctx.enter_context(
                tc.tile_pool(name="psum", bufs=8, space="PSUM"))

            # Issue every input DMA before any output store enters the
            # HWDGE rings: rings are FIFO per issuing engine, so an
            # out-DMA waiting on its PSUM-evacuation copy would head-of-
            # line-block sample 1's weight/feature loads.
            wt_tiles, xp_views = {}, {}
            for b in range(BPC):
                wt = wpool.tile([CI, KS * KS * CO], BF16, tag="wt",
                                name=f"wt{b}")
                if b == 0:
                    # tap-0 slice first: the very first matmul needs only
                    # wt[:, :CO]
                    nc.scalar.dma_start(wt[:, :CO], wts[b][:, :CO])
                    nc.scalar.dma_start(wt[:, CO:], wts[b][:, CO:])
                else:
                    nc.scalar.dma_start(wt[:], wts[b])
                wt_tiles[b] = wt
            splits = np.linspace(0, PADH, FEAT_SPLIT + 1).astype(int)
            for b in range(BPC):
                xp = xpool.tile([CI, PADH * PADW], BF16, tag="xp",
                                name=f"xp{b}")
                xpv = xp[:].rearrange("p (h w) -> p h w", w=PADW)
                fv = feats[b].rearrange("p (h w) -> p h w", w=PADW)
                if b == 0:
                    # 10-row first slice: covers chunk 0's padded rows
                    # 0..9 so its matmuls unblock after ~1/7 of the image
                    bounds = [0, 10] + list(
                        np.linspace(10, PADH, FEAT_SPLIT).astype(int))[1:]
                else:
                    bounds = list(splits)
                for s0, s1 in zip(bounds[:-1], bounds[1:]):
                    # sample 0 -> sync ring, sample 1 -> scalar ring: the
                    # two HWDGE queues stream in parallel (2x aggregate
                    # throughput) and sample 1 is fully resident well
                    # before the PE reaches it
                    eng = nc.sync if b == 0 else nc.scalar
                    eng.dma_start(xpv[:, s0:s1, :], fv[:, s0:s1, :])
                xp_views[b] = xpv

            def emit_chunk(b, wt, xpv, row0, rows, off):
                ps = pspool.tile([CO, rows * W], F32, tag="ps",
                                 name=f"ps{b}_{off}")
                for t in range(KS * KS):
                    kh, kw = divmod(t, KS)
                    rhs = xpv[:, row0 + kh:row0 + kh + rows, kw:kw + W]
                    nc.tensor.matmul(ps[:], wt[:, t * CO:(t + 1) * CO],
                                     rhs, start=(t == 0),
                                     stop=(t == KS * KS - 1))
                ob = opool.tile([CO, rows * W], BF16, tag="ob")
                nc.vector.tensor_copy(ob[:], ps[:])
                # sample-1 stores ride the sync ring (idle once sample-0's
                # features are in): shorter per-ring store queue, so the
                # final store issues sooner after the last matmul
                eng = nc.scalar if b == 0 else nc.sync
                eng.dma_start(out[b][:, off:off + rows * W], ob[:])

            for b in range(BPC):
                wt = wt_tiles[b]
                xpv = xp_views[b]
                for k in range(NCHUNK):
                    if b == BPC - 1 and k == NCHUNK - 1:
                        # split the very last chunk in two: halves the
                        # tail-critical evac+store after the final matmul
                        half = ROWS_PER_CHUNK // 2
                        emit_chunk(b, wt, xpv, ROWS_PER_CHUNK * k, half,
                                   NFREE * k)
                        emit_chunk(b, wt, xpv, ROWS_PER_CHUNK * k + half,
                                   half, NFREE * k + half * W)
                    else:
                        emit_chunk(b, wt, xpv, ROWS_PER_CHUNK * k,
                                   ROWS_PER_CHUNK, NFREE * k)
    nc.compile()
    return nc


def _host_pack_weights(dynamic_kernel):
    import ml_dtypes
    w = np.ascontiguousarray(
        np.asarray(dynamic_kernel).astype(np.float32).transpose(0, 2, 3, 4, 1))
    return w.reshape(B, CI, KS * KS * CO).astype(ml_dtypes.bfloat16)


def _host_pad_features(features):
    import ml_dtypes
    xp = np.zeros((B, CI, PADH, PADW), ml_dtypes.bfloat16)
    xp[:, :, 1:H + 1, 1:W + 1] = np.asarray(features).astype(
        ml_dtypes.bfloat16)
    return xp.reshape(B, CI, PADH * PADW)


def kernel(features, dynamic_kernel):
    """features (16,128,64,64) f32, dynamic_kernel (16,128,128,3,3) f32
    -> (16,128,64,64) f32."""
    from concourse.bass_utils import run_bass_kernel_spmd

    features = np.asarray(features)
    dynamic_kernel = np.asarray(dynamic_kernel)

    if "nc" not in _CACHE:
        _CACHE["nc"] = _build_conv()
    nc = _CACHE["nc"]

    f_padded = _host_pad_features(features)
    w_packed = _host_pack_weights(dynamic_kernel)
    in_maps = [{"features": f_padded[BPC * c:BPC * (c + 1)],
                "weights": w_packed[BPC * c:BPC * (c + 1)]}
               for c in range(N_CORES)]

    import time as _time
    last_err = None
    for attempt in range(4):  # transient NRT/device errors: retry
        try:
            res = run_bass_kernel_spmd(nc, in_maps,
                                       core_ids=list(range(N_CORES)))
            break
        except Exception as e:  # noqa: BLE001
            last_err = e
            # give the terminal time to recover a wedged core before
            # the next attempt (immediate retries hit the same state)
            _time.sleep(5 * (attempt + 1))
    else:
        raise last_err

    got = np.concatenate([res.results[c]["out"] for c in range(N_CORES)],
                         axis=0)
    return got.reshape(B, CO, H, W).astype(np.float32)
